# revision 35
# baseline (speedup 1.0000x reference)
"""MoE layer (N=16384, D=1024, E=8, H=2048, top-2) on 8 trn2 NeuronCores.

Strategy: expert parallelism. The reference computes every expert densely but
only the top-2 survive the gather — so we dispatch each token to its two
routed experts only (4x compute saving). Core c owns expert c's weights; the
host computes the gating (bit-identically to the reference, CPU jax) and
all-to-all-dispatches gathered token batches; each core runs a dense
  y = gelu(x @ W1 + b1) @ W2 + b2
MLP over its batch in bf16 (full PE rate, FWL weight loads that hide behind
the matmul stream, half the DMA bytes of fp32); the host applies the routing
weights and scatter-adds the two expert contributions plus the residual.

All device tensors are packed host-side into the exact SBUF tile layouts so
every DMA moves 2-8KB contiguous runs per partition (strided layouts emit one
descriptor per contiguous line and run descriptor-bound at ~20 GB/s).

Self-contained: only numpy/jax/ml_dtypes/concourse imports.
"""
import numpy as np

import concourse.bass as bass
import concourse.mybir as mybir
import concourse.tile as tile
from concourse.bass_utils import run_bass_kernel_spmd

N, D, E, H, TOP_K = 16384, 1024, 8, 2048, 2
P = 128
CGRAIN = 8       # capacity padding granularity
BMAIN = 512      # main token block (moving dim per matmul)
KD = D // P      # 8 k-tiles over D
JH = H // P      # 16 h-tiles over H
KH = KD // 2
NWARM = 20       # PE warmup matmuls issued while the startup DMAs stream

TRACE = False          # test harness may flip this
TRACE_CORES = None     # e.g. list(range(8)) to profile every core
LAST_RESULTS = None    # BassKernelResults of the last device run

F32 = mybir.dt.float32
BF16 = mybir.dt.bfloat16


def _split_excess_waits(nc, max_waits=1):
    """This walrus build rejects >1 sem-wait per instruction; Tile emits more.
    Move excess waits onto same-engine NOPs inserted right before."""
    for fn in nc.m.functions:
        for blk in fn.blocks:
            insts = list(blk.instructions)
            out = []
            changed = False
            for inst in insts:
                si = getattr(inst, "sync_info", None)
                if si is not None and si.on_wait and len(si.on_wait) > max_waits:
                    waits = list(si.on_wait)
                    excess, keep = waits[:-max_waits], waits[-max_waits:]
                    for i in range(0, len(excess), max_waits):
                        out.append(
                            mybir.InstNoOp(
                                name=nc.get_next_instruction_name(),
                                engine=inst.engine,
                                sync_info=mybir.SyncInfo(
                                    on_wait=excess[i : i + max_waits], on_update=[]
                                ),
                                bass_nofuse=True,
                            )
                        )
                    inst.sync_info = mybir.SyncInfo(
                        on_wait=keep, on_update=list(si.on_update)
                    )
                    changed = True
                out.append(inst)
            if changed:
                blk.instructions = out


def _plan_blocks(C):
    """512-wide blocks; a sub-512 remainder becomes one block in [256,512] or
    two (rem-256, 256) blocks so every matmul keeps an efficient moving dim.
    Tail blocks go last: a narrow first block would pull the weight-arrival
    deadlines into the startup DMA burst, and a small final block shortens
    the end-of-kernel drain."""
    blocks, off = [], 0
    while C - off >= BMAIN + 256:
        blocks.append((off, BMAIN))
        off += BMAIN
    rem = C - off
    if rem > BMAIN:
        blocks.append((off, rem - 256))
        blocks.append((off + rem - 256, 256))
    elif rem:
        blocks.append((off, rem))
    return blocks


def build_nc(C: int):
    """Per-core dense expert MLP: y = gelu(x @ w1 + b1) @ w2 + b2, all
    operands pre-packed into SBUF tile layout (partition-contiguous)."""
    nc = bass.Bass("TRN2", target_bir_lowering=False)
    xpk = nc.dram_tensor("xpk", (P, KD * C), BF16, kind="ExternalInput")
    w1pk = nc.dram_tensor("w1pk", (P, JH * KD * P), BF16, kind="ExternalInput")
    b1v = nc.dram_tensor("b1v", (P, JH), F32, kind="ExternalInput")
    w2pk = nc.dram_tensor("w2pk", (P, KD * JH * P), BF16, kind="ExternalInput")
    b2v = nc.dram_tensor("b2v", (P, KD), F32, kind="ExternalInput")
    ypk = nc.dram_tensor("ypk", (P, KD * C), BF16, kind="ExternalOutput")

    with tile.TileContext(nc) as tc:
        with (
            tc.tile_pool(name="wpool", bufs=1) as wpool,
            tc.tile_pool(name="xpool", bufs=2) as xpool,
            tc.tile_pool(name="hpool", bufs=2) as hpool,
            tc.tile_pool(name="ypool", bufs=2) as ypool,
            tc.tile_pool(name="psum", bufs=4, space="PSUM") as psum,
        ):
            blocks = _plan_blocks(C)

            # PE warmup: matmuls on a memset tile with no DMA dependencies.
            # They run during the startup DMA burst so the HAM clock gate is
            # already at 8/8 when the first real matmul issues.
            wzero = wpool.tile([P, 256], BF16, name="wzero")
            nc.gpsimd.memset(wzero[:], 0.0)
            pwarm = psum.tile([P, BMAIN], F32, tag="ph")
            for _ in range(NWARM):
                nc.tensor.matmul(
                    pwarm[:, :256], wzero[:, :P], wzero[:], start=True, stop=True
                )

            def load_block(off, B, eng):
                # two half-tiles: the first matmul chain waits on 0.5MB, not 1MB
                xa = xpool.tile([P, KH, B], BF16, tag="xa")
                eng.dma_start(xa[:], xpk[:, KD * off : KD * off + KH * B])
                xc = xpool.tile([P, KH, B], BF16, tag="xc")
                eng.dma_start(xc[:], xpk[:, KD * off + KH * B : KD * (off + B)])
                return xa, xc

            # w1 for the startup-critical j<8 lives as k-half tiles so the
            # first chains' deadlines are 128KB apart instead of 256KB —
            # keeps every early stall under the HAM re-throttle window.
            NSPLIT = 8
            w1a = [wpool.tile([P, KH, P], BF16, tag=f"w1a_{j}", name=f"w1a_{j}") for j in range(NSPLIT)]
            w1c = [wpool.tile([P, KH, P], BF16, tag=f"w1c_{j}", name=f"w1c_{j}") for j in range(NSPLIT)]
            w1sb = [
                wpool.tile([P, KD, P], BF16, tag=f"w1_{j}", name=f"w1_{j}")
                if j >= NSPLIT
                else None
                for j in range(JH)
            ]
            w2sb = [wpool.tile([P, JH, P], BF16, tag=f"w2_{d}", name=f"w2_{d}") for d in range(KD)]

            def w1ap(j, k):
                if j < NSPLIT:
                    return w1a[j][:, k] if k < KH else w1c[j][:, k - KH]
                return w1sb[j][:, k]

            def load_w1(j, eng, half=None):
                if j >= NSPLIT:
                    eng.dma_start(w1sb[j][:], w1pk[:, j * KD * P : (j + 1) * KD * P])
                    return
                base = j * KD * P + (half or 0) * KH * P
                t = w1a[j] if not half else w1c[j]
                eng.dma_start(t[:], w1pk[:, base : base + KH * P])

            # DMA paths (three queues sharing ~358GB/s of HBM). The sync
            # HWDGE ring is the fastest starter, so it carries the startup
            # critical path in deadline order; the other two rings are held
            # back with cheap engine work (~1.7us per memset/copy) so they
            # don't steal HBM bandwidth from it — their own deadlines are
            # ~10-25us later.
            #  - sync: w1_0, block-0 x, w1_1..3, then the token stream
            #  - scalar: b1, delay, w1_4..7, then the y writebacks
            #  - gpsimd SWDGE: delay, then the 4.2MB w1-tail/w2 bulk
            b1sb = wpool.tile([P, JH], F32)
            b2sb = wpool.tile([P, KD], F32)
            load_w1(0, nc.sync, half=0)
            xa0, xc0 = load_block(*blocks[0], nc.sync)
            load_w1(0, nc.sync, half=1)
            for j in range(1, 4):
                load_w1(j, nc.sync, half=0)
                load_w1(j, nc.sync, half=1)

            dly = wpool.tile([P, 2048], F32, name="dly")
            nc.gpsimd.memset(dly[:], 0.0)
            nc.scalar.dma_start(b1sb[:], b1v[:])
            for _ in range(2):
                nc.scalar.activation(
                    dly[:], dly[:], mybir.ActivationFunctionType.Copy
                )
            for j in range(4, 8):
                load_w1(j, nc.scalar, half=0)
                load_w1(j, nc.scalar, half=1)
            nc.scalar.dma_start(b2sb[:], b2v[:])

            dly2 = wpool.tile([P, 2048], F32, name="dly2")
            for _ in range(2):
                nc.gpsimd.memset(dly2[:], 0.0)
            for j in range(8, JH):
                load_w1(j, nc.gpsimd)
            for d in range(KD):
                nc.gpsimd.dma_start(w2sb[d][:], w2pk[:, d * JH * P : (d + 1) * JH * P])

            for bi, (off, B) in enumerate(blocks):
                if bi == 0:
                    xa, xc = xa0, xc0
                else:
                    xa, xc = load_block(off, B, nc.sync)
                hb = hpool.tile([P, JH, B], BF16, tag="hb")
                # h^T[j] = gelu(W1[:, j].T @ x^T + b1[j])
                for j in range(JH):
                    ph = psum.tile([P, B], F32, tag="ph")
                    for k in range(KD):
                        nc.tensor.matmul(
                            ph[:],
                            w1ap(j, k),
                            xa[:, k] if k < KH else xc[:, k - KH],
                            start=(k == 0),
                            stop=(k == KD - 1),
                        )
                    nc.scalar.activation(
                        hb[:, j],
                        ph[:],
                        mybir.ActivationFunctionType.Gelu,
                        bias=b1sb[:, j : j + 1],
                    )
                # y^T[d] = W2[:, d].T @ h^T + b2[d]
                yst = ypool.tile([P, KD, B], BF16, tag="yst")
                last = bi == len(blocks) - 1
                for d in range(KD):
                    pd = psum.tile([P, B], F32, tag="pd")
                    for j in range(JH):
                        nc.tensor.matmul(
                            pd[:],
                            w2sb[d][:, j],
                            hb[:, j],
                            start=(j == 0),
                            stop=(j == JH - 1),
                        )
                    nc.scalar.activation(
                        yst[:, d],
                        pd[:],
                        mybir.ActivationFunctionType.Identity,
                        bias=b2sb[:, d : d + 1],
                    )
                    if last and d == KD // 2 - 1:
                        # flush the first half early so the end-of-kernel
                        # barrier only waits on a 0.25MB transfer
                        nc.scalar.dma_start(
                            ypk[:, KD * off : KD * off + KH * B], yst[:, :KH]
                        )
                if last:
                    nc.scalar.dma_start(
                        ypk[:, KD * off + KH * B : KD * (off + B)], yst[:, KH:]
                    )
                else:
                    nc.scalar.dma_start(ypk[:, KD * off : KD * (off + B)], yst[:])
    _split_excess_waits(nc)
    return nc


_NC_CACHE = {}


def _routing(x, Wg, bg):
    """Gating computed the same way (and on the same platform: CPU jax) as the
    reference, so the top-2 choice is bit-identical even for near-tie logits."""
    import jax
    import jax.numpy as jnp

    cpu = jax.local_devices(backend="cpu")[0]
    with jax.default_device(cpu):
        logits = jnp.asarray(x) @ jnp.asarray(Wg) + jnp.asarray(bg)
        probs = jax.nn.softmax(logits, axis=-1)
        topk_p, topk_i = jax.lax.top_k(probs, TOP_K)
        topk_p = topk_p / topk_p.sum(axis=-1, keepdims=True)
    return np.asarray(topk_i), np.asarray(topk_p)


def _pack_x(xg, C, blocks, bf16):
    """xg (C, D) -> (P, KD*C): per block, k-major then token-major, so each
    xa/xc DMA reads one contiguous 2-4KB run per partition."""
    x3 = np.asarray(xg, dtype=bf16).reshape(C, KD, P)
    parts = [
        np.transpose(x3[off : off + B], (2, 1, 0)).reshape(P, KD * B)
        for off, B in blocks
    ]
    return np.ascontiguousarray(np.concatenate(parts, axis=1))


def _unpack_y(ypk, C, blocks):
    """(P, KD*C) bf16 -> (C, D) fp32, inverse of the yst tile layout."""
    y = np.empty((C, D), np.float32)
    for off, B in blocks:
        blk = ypk[:, KD * off : KD * (off + B)].reshape(P, KD, B)
        y[off : off + B] = np.transpose(blk, (2, 1, 0)).reshape(B, D)
    return y


def kernel(x, Wg, bg, W1, b1, W2, b2):
    global LAST_RESULTS
    import ml_dtypes

    bf16 = ml_dtypes.bfloat16
    x = np.ascontiguousarray(np.asarray(x, dtype=np.float32))
    Wg = np.asarray(Wg, dtype=np.float32)
    bg = np.asarray(bg, dtype=np.float32)
    W1 = np.asarray(W1, dtype=np.float32)
    b1 = np.asarray(b1, dtype=np.float32)
    W2 = np.asarray(W2, dtype=np.float32)
    b2 = np.asarray(b2, dtype=np.float32)

    topk_i, topk_p = _routing(x, Wg, bg)

    idx_list, p_list = [], []
    for e in range(E):
        m0 = topk_i[:, 0] == e
        m1 = topk_i[:, 1] == e
        idx = np.nonzero(m0 | m1)[0]
        p = np.where(m0[idx], topk_p[idx, 0], topk_p[idx, 1]).astype(np.float32)
        idx_list.append(idx)
        p_list.append(p)

    # Capacity limiting at factor 1.0: each expert takes at most N*TOP_K/E
    # tokens on-device; the overflow pairs (lowest routing weight first,
    # 0.8% of pairs for the reference distribution) are computed exactly on
    # the host during the gather/scatter pass. Keeps every core at exactly
    # 8 full 512-wide blocks.
    CAP = N * TOP_K // E
    overflow = []
    for e in range(E):
        n = len(idx_list[e])
        if n > CAP:
            order = np.argsort(p_list[e])
            spill, keep = order[: n - CAP], np.sort(order[n - CAP :])
            overflow.append((e, idx_list[e][spill], p_list[e][spill]))
            idx_list[e] = idx_list[e][keep]
            p_list[e] = p_list[e][keep]

    cmax = max(len(i) for i in idx_list)
    C = max(256, ((cmax + CGRAIN - 1) // CGRAIN) * CGRAIN)
    blocks = _plan_blocks(C)

    if C not in _NC_CACHE:
        _NC_CACHE[C] = build_nc(C)
    nc = _NC_CACHE[C]

    in_maps = []
    for e in range(E):
        idx = idx_list[e]
        n = len(idx)
        xg = np.zeros((C, D), np.float32)
        xg[:n] = x[idx]
        # w1pk[p, j, k, q] = W1[e][k*P+p, j*P+q]; w2pk[p, d, j, q] = W2[e][j*P+p, d*P+q]
        w1p = np.transpose(
            np.asarray(W1[e], dtype=bf16).reshape(KD, P, JH, P), (1, 2, 0, 3)
        ).reshape(P, JH * KD * P)
        w2p = np.transpose(
            np.asarray(W2[e], dtype=bf16).reshape(JH, P, KD, P), (1, 2, 0, 3)
        ).reshape(P, KD * JH * P)
        in_maps.append(
            {
                "xpk": _pack_x(xg, C, blocks, bf16),
                "w1pk": np.ascontiguousarray(w1p),
                "b1v": np.ascontiguousarray(b1[e].reshape(JH, P).T),
                "w2pk": np.ascontiguousarray(w2p),
                "b2v": np.ascontiguousarray(b2[e].reshape(KD, P).T),
            }
        )

    res = run_bass_kernel_spmd(
        nc, in_maps, core_ids=list(range(E)), trace=TRACE, trace_cores=TRACE_CORES
    )
    LAST_RESULTS = res

    out = x.copy()
    for e in range(E):
        idx = idx_list[e]
        ye = _unpack_y(np.asarray(res.results[e]["ypk"], np.float32), C, blocks)
        out[idx] += ye[: len(idx)] * p_list[e][:, None]
    if overflow:
        import jax
        import jax.numpy as jnp

        cpu = jax.local_devices(backend="cpu")[0]
        with jax.default_device(cpu):
            for e, didx, dp in overflow:
                h = jax.nn.gelu(
                    jnp.asarray(x[didx]) @ jnp.asarray(W1[e]) + b1[e],
                    approximate=False,
                )
                ye = np.asarray(h @ jnp.asarray(W2[e]) + b2[e])
                out[didx] += ye * dp[:, None]
    return out


# revision 40
# speedup vs baseline: 1.0143x; 1.0143x over previous
"""MoE layer (N=16384, D=1024, E=8, H=2048, top-2) on 8 trn2 NeuronCores.

Strategy: expert parallelism. The reference computes every expert densely but
only the top-2 survive the gather — so we dispatch each token to its two
routed experts only (4x compute saving). Core c owns expert c's weights; the
host computes the gating (bit-identically to the reference, CPU jax) and
all-to-all-dispatches gathered token batches; each core runs a dense
  y = gelu(x @ W1 + b1) @ W2 + b2
MLP over its batch in bf16 (full PE rate, FWL weight loads that hide behind
the matmul stream, half the DMA bytes of fp32); the host applies the routing
weights and scatter-adds the two expert contributions plus the residual.

All device tensors are packed host-side into the exact SBUF tile layouts so
every DMA moves 2-8KB contiguous runs per partition (strided layouts emit one
descriptor per contiguous line and run descriptor-bound at ~20 GB/s).

Self-contained: only numpy/jax/ml_dtypes/concourse imports.
"""
import numpy as np

import concourse.bass as bass
import concourse.mybir as mybir
import concourse.tile as tile
from concourse.bass_utils import run_bass_kernel_spmd

N, D, E, H, TOP_K = 16384, 1024, 8, 2048, 2
P = 128
CGRAIN = 8       # capacity padding granularity
BMAIN = 512      # main token block (moving dim per matmul)
KD = D // P      # 8 k-tiles over D
JH = H // P      # 16 h-tiles over H
KH = KD // 2
NWARM = 28       # PE warmup matmuls issued while the startup DMAs stream

TRACE = False          # test harness may flip this
TRACE_CORES = None     # e.g. list(range(8)) to profile every core
LAST_RESULTS = None    # BassKernelResults of the last device run

F32 = mybir.dt.float32
BF16 = mybir.dt.bfloat16


def _split_excess_waits(nc, max_waits=1):
    """This walrus build rejects >1 sem-wait per instruction; Tile emits more.
    Move excess waits onto same-engine NOPs inserted right before."""
    for fn in nc.m.functions:
        for blk in fn.blocks:
            insts = list(blk.instructions)
            out = []
            changed = False
            for inst in insts:
                si = getattr(inst, "sync_info", None)
                if si is not None and si.on_wait and len(si.on_wait) > max_waits:
                    waits = list(si.on_wait)
                    excess, keep = waits[:-max_waits], waits[-max_waits:]
                    for i in range(0, len(excess), max_waits):
                        out.append(
                            mybir.InstNoOp(
                                name=nc.get_next_instruction_name(),
                                engine=inst.engine,
                                sync_info=mybir.SyncInfo(
                                    on_wait=excess[i : i + max_waits], on_update=[]
                                ),
                                bass_nofuse=True,
                            )
                        )
                    inst.sync_info = mybir.SyncInfo(
                        on_wait=keep, on_update=list(si.on_update)
                    )
                    changed = True
                out.append(inst)
            if changed:
                blk.instructions = out


def _plan_blocks(C):
    """512-wide blocks; a sub-512 remainder becomes one block in [256,512] or
    two (rem-256, 256) blocks so every matmul keeps an efficient moving dim.
    Tail blocks go last: a narrow first block would pull the weight-arrival
    deadlines into the startup DMA burst, and a small final block shortens
    the end-of-kernel drain."""
    blocks, off = [], 0
    while C - off >= BMAIN + 256:
        blocks.append((off, BMAIN))
        off += BMAIN
    rem = C - off
    if rem > BMAIN:
        blocks.append((off, rem - 256))
        blocks.append((off + rem - 256, 256))
    elif rem:
        blocks.append((off, rem))
    return blocks


def build_nc(C: int):
    """Per-core dense expert MLP: y = gelu(x @ w1 + b1) @ w2 + b2, all
    operands pre-packed into SBUF tile layout (partition-contiguous)."""
    nc = bass.Bass("TRN2", target_bir_lowering=False)
    xpk = nc.dram_tensor("xpk", (P, KD * C), BF16, kind="ExternalInput")
    w1pk = nc.dram_tensor("w1pk", (P, JH * KD * P), BF16, kind="ExternalInput")
    b1v = nc.dram_tensor("b1v", (P, JH), F32, kind="ExternalInput")
    w2pk = nc.dram_tensor("w2pk", (P, KD * JH * P), BF16, kind="ExternalInput")
    b2v = nc.dram_tensor("b2v", (P, KD), F32, kind="ExternalInput")
    ypk = nc.dram_tensor("ypk", (P, KD * C), BF16, kind="ExternalOutput")

    with tile.TileContext(nc) as tc:
        with (
            tc.tile_pool(name="wpool", bufs=1) as wpool,
            tc.tile_pool(name="xpool", bufs=2) as xpool,
            tc.tile_pool(name="hpool", bufs=2) as hpool,
            tc.tile_pool(name="ypool", bufs=2) as ypool,
            tc.tile_pool(name="psum", bufs=4, space="PSUM") as psum,
        ):
            blocks = _plan_blocks(C)

            # PE warmup: matmuls on a memset tile with no DMA dependencies.
            # They run during the startup DMA burst so the HAM clock gate is
            # already at 8/8 when the first real matmul issues.
            wzero = wpool.tile([P, 256], BF16, name="wzero")
            nc.gpsimd.memset(wzero[:], 0.0)
            pwarm = psum.tile([P, BMAIN], F32, tag="ph")
            for _ in range(NWARM):
                nc.tensor.matmul(
                    pwarm[:, :256], wzero[:, :P], wzero[:], start=True, stop=True
                )

            def load_block(off, B, eng):
                # two half-tiles: the first matmul chain waits on 0.5MB, not 1MB
                xa = xpool.tile([P, KH, B], BF16, tag="xa")
                eng.dma_start(xa[:], xpk[:, KD * off : KD * off + KH * B])
                xc = xpool.tile([P, KH, B], BF16, tag="xc")
                eng.dma_start(xc[:], xpk[:, KD * off + KH * B : KD * (off + B)])
                return xa, xc

            w1sb = [wpool.tile([P, KD, P], BF16, tag=f"w1_{j}", name=f"w1_{j}") for j in range(JH)]
            w2sb = [wpool.tile([P, JH, P], BF16, tag=f"w2_{d}", name=f"w2_{d}") for d in range(KD)]

            def load_w1(j, eng):
                eng.dma_start(w1sb[j][:], w1pk[:, j * KD * P : (j + 1) * KD * P])

            # DMA paths (three queues sharing ~358GB/s of HBM). The sync
            # HWDGE ring is the fastest starter, so it carries the startup
            # critical path in deadline order; the other two rings are held
            # back with cheap engine work (~1.7us per memset/copy) so they
            # don't steal HBM bandwidth from it — their own deadlines are
            # ~10-25us later.
            #  - sync: w1_0, block-0 x, w1_1..3, then the token stream
            #  - scalar: b1, delay, w1_4..7, then the y writebacks
            #  - gpsimd SWDGE: delay, then the 4.2MB w1-tail/w2 bulk
            b1sb = wpool.tile([P, JH], F32)
            b2sb = wpool.tile([P, KD], F32)
            load_w1(0, nc.sync)
            xa0, xc0 = load_block(*blocks[0], nc.sync)
            load_w1(1, nc.sync)
            load_w1(2, nc.sync)
            load_w1(3, nc.sync)

            dly = wpool.tile([P, 2048], F32, name="dly")
            nc.gpsimd.memset(dly[:], 0.0)
            nc.scalar.dma_start(b1sb[:], b1v[:])
            for _ in range(2):
                nc.scalar.activation(
                    dly[:], dly[:], mybir.ActivationFunctionType.Copy
                )
            for j in range(4, 8):
                load_w1(j, nc.scalar)
            nc.scalar.dma_start(b2sb[:], b2v[:])

            dly2 = wpool.tile([P, 2048], F32, name="dly2")
            for _ in range(2):
                nc.gpsimd.memset(dly2[:], 0.0)
            for j in range(8, JH):
                load_w1(j, nc.gpsimd)
            for d in range(KD):
                nc.gpsimd.dma_start(w2sb[d][:], w2pk[:, d * JH * P : (d + 1) * JH * P])

            for bi, (off, B) in enumerate(blocks):
                if bi == 0:
                    xa, xc = xa0, xc0
                else:
                    xa, xc = load_block(off, B, nc.sync)
                hb = hpool.tile([P, JH, B], BF16, tag="hb")
                # h^T[j] = gelu(W1[:, j].T @ x^T + b1[j])
                for j in range(JH):
                    ph = psum.tile([P, B], F32, tag="ph")
                    for k in range(KD):
                        nc.tensor.matmul(
                            ph[:],
                            w1sb[j][:, k],
                            xa[:, k] if k < KH else xc[:, k - KH],
                            start=(k == 0),
                            stop=(k == KD - 1),
                        )
                    nc.scalar.activation(
                        hb[:, j],
                        ph[:],
                        mybir.ActivationFunctionType.Gelu,
                        bias=b1sb[:, j : j + 1],
                    )
                # y^T[d] = W2[:, d].T @ h^T + b2[d]
                yst = ypool.tile([P, KD, B], BF16, tag="yst")
                last = bi == len(blocks) - 1
                for d in range(KD):
                    pd = psum.tile([P, B], F32, tag="pd")
                    for j in range(JH):
                        nc.tensor.matmul(
                            pd[:],
                            w2sb[d][:, j],
                            hb[:, j],
                            start=(j == 0),
                            stop=(j == JH - 1),
                        )
                    nc.scalar.activation(
                        yst[:, d],
                        pd[:],
                        mybir.ActivationFunctionType.Identity,
                        bias=b2sb[:, d : d + 1],
                    )
                    if last and d == KD // 2 - 1:
                        # flush the first half early so the end-of-kernel
                        # barrier only waits on a 0.25MB transfer
                        nc.scalar.dma_start(
                            ypk[:, KD * off : KD * off + KH * B], yst[:, :KH]
                        )
                if last:
                    nc.scalar.dma_start(
                        ypk[:, KD * off + KH * B : KD * (off + B)], yst[:, KH:]
                    )
                else:
                    nc.scalar.dma_start(ypk[:, KD * off : KD * (off + B)], yst[:])
    _split_excess_waits(nc)
    return nc


_NC_CACHE = {}


def _routing(x, Wg, bg):
    """Gating computed the same way (and on the same platform: CPU jax) as the
    reference, so the top-2 choice is bit-identical even for near-tie logits."""
    import jax
    import jax.numpy as jnp

    cpu = jax.local_devices(backend="cpu")[0]
    with jax.default_device(cpu):
        logits = jnp.asarray(x) @ jnp.asarray(Wg) + jnp.asarray(bg)
        probs = jax.nn.softmax(logits, axis=-1)
        topk_p, topk_i = jax.lax.top_k(probs, TOP_K)
        topk_p = topk_p / topk_p.sum(axis=-1, keepdims=True)
    return np.asarray(topk_i), np.asarray(topk_p)


def _pack_x(xg, C, blocks, bf16):
    """xg (C, D) -> (P, KD*C): per block, k-major then token-major, so each
    xa/xc DMA reads one contiguous 2-4KB run per partition."""
    x3 = np.asarray(xg, dtype=bf16).reshape(C, KD, P)
    parts = [
        np.transpose(x3[off : off + B], (2, 1, 0)).reshape(P, KD * B)
        for off, B in blocks
    ]
    return np.ascontiguousarray(np.concatenate(parts, axis=1))


def _unpack_y(ypk, C, blocks):
    """(P, KD*C) bf16 -> (C, D) fp32, inverse of the yst tile layout."""
    y = np.empty((C, D), np.float32)
    for off, B in blocks:
        blk = ypk[:, KD * off : KD * (off + B)].reshape(P, KD, B)
        y[off : off + B] = np.transpose(blk, (2, 1, 0)).reshape(B, D)
    return y


def kernel(x, Wg, bg, W1, b1, W2, b2):
    global LAST_RESULTS
    import ml_dtypes

    bf16 = ml_dtypes.bfloat16
    x = np.ascontiguousarray(np.asarray(x, dtype=np.float32))
    Wg = np.asarray(Wg, dtype=np.float32)
    bg = np.asarray(bg, dtype=np.float32)
    W1 = np.asarray(W1, dtype=np.float32)
    b1 = np.asarray(b1, dtype=np.float32)
    W2 = np.asarray(W2, dtype=np.float32)
    b2 = np.asarray(b2, dtype=np.float32)

    topk_i, topk_p = _routing(x, Wg, bg)

    idx_list, p_list = [], []
    for e in range(E):
        m0 = topk_i[:, 0] == e
        m1 = topk_i[:, 1] == e
        idx = np.nonzero(m0 | m1)[0]
        p = np.where(m0[idx], topk_p[idx, 0], topk_p[idx, 1]).astype(np.float32)
        idx_list.append(idx)
        p_list.append(p)

    # Capacity limiting at factor 1.0: each expert takes at most N*TOP_K/E
    # tokens on-device; the overflow pairs (lowest routing weight first,
    # 0.8% of pairs for the reference distribution) are computed exactly on
    # the host during the gather/scatter pass. Keeps every core at exactly
    # 8 full 512-wide blocks.
    CAP = N * TOP_K // E
    overflow = []
    for e in range(E):
        n = len(idx_list[e])
        if n > CAP:
            order = np.argsort(p_list[e])
            spill, keep = order[: n - CAP], np.sort(order[n - CAP :])
            overflow.append((e, idx_list[e][spill], p_list[e][spill]))
            idx_list[e] = idx_list[e][keep]
            p_list[e] = p_list[e][keep]

    cmax = max(len(i) for i in idx_list)
    C = max(256, ((cmax + CGRAIN - 1) // CGRAIN) * CGRAIN)
    blocks = _plan_blocks(C)

    if C not in _NC_CACHE:
        _NC_CACHE[C] = build_nc(C)
    nc = _NC_CACHE[C]

    in_maps = []
    for e in range(E):
        idx = idx_list[e]
        n = len(idx)
        xg = np.zeros((C, D), np.float32)
        xg[:n] = x[idx]
        # w1pk[p, j, k, q] = W1[e][k*P+p, j*P+q]; w2pk[p, d, j, q] = W2[e][j*P+p, d*P+q]
        w1p = np.transpose(
            np.asarray(W1[e], dtype=bf16).reshape(KD, P, JH, P), (1, 2, 0, 3)
        ).reshape(P, JH * KD * P)
        w2p = np.transpose(
            np.asarray(W2[e], dtype=bf16).reshape(JH, P, KD, P), (1, 2, 0, 3)
        ).reshape(P, KD * JH * P)
        in_maps.append(
            {
                "xpk": _pack_x(xg, C, blocks, bf16),
                "w1pk": np.ascontiguousarray(w1p),
                "b1v": np.ascontiguousarray(b1[e].reshape(JH, P).T),
                "w2pk": np.ascontiguousarray(w2p),
                "b2v": np.ascontiguousarray(b2[e].reshape(KD, P).T),
            }
        )

    res = run_bass_kernel_spmd(
        nc, in_maps, core_ids=list(range(E)), trace=TRACE, trace_cores=TRACE_CORES
    )
    LAST_RESULTS = res

    out = x.copy()
    for e in range(E):
        idx = idx_list[e]
        ye = _unpack_y(np.asarray(res.results[e]["ypk"], np.float32), C, blocks)
        out[idx] += ye[: len(idx)] * p_list[e][:, None]
    if overflow:
        import jax
        import jax.numpy as jnp

        cpu = jax.local_devices(backend="cpu")[0]
        with jax.default_device(cpu):
            for e, didx, dp in overflow:
                h = jax.nn.gelu(
                    jnp.asarray(x[didx]) @ jnp.asarray(W1[e]) + b1[e],
                    approximate=False,
                )
                ye = np.asarray(h @ jnp.asarray(W2[e]) + b2[e])
                out[didx] += ye * dp[:, None]
    return out
